# revision 24
# baseline (speedup 1.0000x reference)
"""Causal attention kernel for Trainium2 (Bass/Tile), 8-core SPMD.

Problem: B=16, S=2048, D=128 fp32 causal attention
    scores = Q @ K^T; scores -= INF*triu(k=1); attn = softmax(scores/sqrt(D));
    out = attn @ V.   Batch dim sharded across 8 cores, 2 batches per core.

v3 design (from v2 @ ~59.5us):
  - Startup: qkv piece-0 DMA issued first on the sync ring (cold DMA latency
    ~2.5us dominates the head); all remaining loads queue behind it in ring
    order (in-order per-queue service gives each piece near-solo bandwidth,
    replacing v2's ACT-ring burn-activation pacing).  Warmup is 20 x 128-col
    matmuls (fine-grained: blocks the first real MM by <=110ns) plus a 1-col
    exp to hoist the ACT table load.
  - Full (non-diagonal) k-chunk groups go fp8 end-to-end: exp writes
    float8e4 P' directly (no fp16 copy, no DVE cast), and BOTH the PV
    matmul and the rowsum run as fp8 DoubleRow (contract 256 = 2 chunks
    per 512-col stream): PE stream work drops ~5us/core vs fp16 PV.
    V rides a second fp8 input tensor (chunk-pair interleaved to match
    the DR weight AP).  Numerics: P and l use the same quantized P', so
    fp8 error largely cancels in O/l; numpy sim of this exact pipeline
    measures rel err 6.3e-3 (gate 2e-2).
  - exp is split across two engines: ACT (LUT exp) takes 7 of 12 full
    groups per batch + all diagonal groups; the DVE takes 5 via a
    Schraudolph exp2 bit-trick (one fused mul-add tensor_scalar fp32->int32
    PSUM->SBUF, then a bitcast fp32->fp8 copy).  Without this the ACT is
    the bottleneck at ~41us busy; split, both sit ~29us under the PE's
    ~34us.  Trick groups are always full groups (>=512 attended keys per
    row), where the ~4% sawtooth error cancels between numerator and
    denominator (sim: no measurable degradation over pure fp8).
  - Diagonal groups keep exact fp16 exp + fp16 PV/rowsum (rows with few
    attended keys can't absorb fp8/approx error), masked via GPSIMD
    affine_select after the exp as in v2.
  - l is shipped per q-block (8 small DMAs) instead of one big end-of-
    kernel DMA; PV emission is deferred two groups behind exp so the
    in-order PE queue always has QK work while both exp engines run.
Host: out = (O^T / l).T in fp64.
"""

import os

os.environ.setdefault("MYCRO_LOCAL_CACHE", "1")

import math

import numpy as np
import ml_dtypes

import concourse.bass as bass
import concourse.mybir as mybir
import concourse.tile as tile
from concourse import bacc
from concourse.bass_utils import run_bass_kernel_spmd

F32 = mybir.dt.float32
F16 = mybir.dt.float16
F8 = mybir.dt.float8e4
I32 = mybir.dt.int32
EXPF = mybir.ActivationFunctionType.Exp
DR = mybir.MatmulPerfMode.DoubleRow
MULT = mybir.AluOpType.mult
ADD = mybir.AluOpType.add

N_CORES = 8
B = 16
S = 2048
D = 128
BPC = B // N_CORES
SCALE = 1.0 / math.sqrt(float(D))
SHIFT = -2.0  # exp bias; cancels in O/l, keeps P' <= e^4 fp8-safe
NQB = S // 512
NCH = S // 128

LOG2E = 1.4426950408889634
TRICK_C = 0.0579  # Schraudolph bias: minimizes max rel err of 2^frac approx
TRICK_A = SCALE * LOG2E * (1 << 23)
TRICK_B = ((127.0 - TRICK_C) + SHIFT * LOG2E) * (1 << 23)

N_WARM_MM = 28

# packed qkv column map, 4 pieces per batch (one per q-block of demand):
#   piece i [1536*i : 1536*(i+1)] = k[512i:512(i+1)] | q[...] | v[...]
# piece 0 is special-cased as k[0:256] | q[0:512] | k[256:512] | v[0:512] so
# the head DMA [0:768] carries exactly the first diagonal group's operands.


def _kt_col(c):
    if c < 256:
        return c
    if c < 512:
        return 768 + (c - 256)
    return 1536 * (c // 512) + (c % 512)


def _qt_col(c):
    if c < 512:
        return 256 + c
    return 1536 * (c // 512) + 512 + (c % 512)


def _vr_col(c):
    return 1536 * (c // 512) + 1024 + (c % 512)


def build():
    nc = bacc.Bacc("TRN2", target_bir_lowering=False, debug=False, num_devices=N_CORES)
    x_d = nc.dram_tensor("x", [BPC, 128, 6144], F16, kind="ExternalInput")
    x8_d = nc.dram_tensor("x8", [BPC, 128, 1536], F8, kind="ExternalInput")
    o_d = nc.dram_tensor("o", [BPC, 128, S], F16, kind="ExternalOutput")
    l_d = nc.dram_tensor("l", [1, BPC * NQB * 512], F32, kind="ExternalOutput")

    with tile.TileContext(nc) as tc:
        with (
            tc.tile_pool(name="const", bufs=1) as constp,
            tc.tile_pool(name="qkv", bufs=2) as qkvp,
            tc.tile_pool(name="v8", bufs=2) as v8p,
            tc.tile_pool(name="pt", bufs=4) as ptp,
            tc.tile_pool(name="pt8", bufs=4) as pt8p,
            tc.tile_pool(name="trick", bufs=3) as trickp,
            tc.tile_pool(name="evac", bufs=3) as evacp,
            tc.tile_pool(name="lsb", bufs=2) as lsbp,
            tc.tile_pool(name="stps", bufs=3, space="PSUM") as stps,
            tc.tile_pool(name="otps", bufs=1, space="PSUM") as otps,
            tc.tile_pool(name="lps", bufs=1, space="PSUM") as lps,
        ):
            # ---- DMA-first startup: piece0 rides the cold ring latency ----
            qkv_tiles = [
                qkvp.tile([128, 6144], F16, name=f"qkv{b}") for b in range(BPC)
            ]
            v8_tiles = [
                v8p.tile([128, 1536], F8, name=f"v8_{b}") for b in range(BPC)
            ]
            nc.sync.dma_start(qkv_tiles[0][:, 0:768], x_d[0, :, 0:768])
            nc.sync.dma_start(qkv_tiles[0][:, 768:1536], x_d[0, :, 768:1536])
            nc.sync.dma_start(qkv_tiles[0][:, 1536:3072], x_d[0, :, 1536:3072])
            nc.sync.dma_start(v8_tiles[0][:], x8_d[0, :, :])
            nc.sync.dma_start(qkv_tiles[0][:, 3072:6144], x_d[0, :, 3072:6144])
            if BPC > 1:
                nc.sync.dma_start(qkv_tiles[1][:, 0:3072], x_d[1, :, 0:3072])
                nc.sync.dma_start(qkv_tiles[1][:, 3072:6144], x_d[1, :, 3072:6144])
                nc.sync.dma_start(v8_tiles[1][:], x8_d[1, :, :])

            # ---- warmup: ACT table load + PE HAM ramp while DMAs fly ----
            # dummy is written by the DVE (whose post-boot user slot opens
            # ~1us before GPSIMD's) so the warmup matmul streak — and with
            # it the HAM busy-window credit — starts as early as possible.
            dummy = constp.tile([128, 128], F16, name="dummy")
            nc.vector.memset(dummy[:], 0.0)
            shiftb = constp.tile([128, 1], F32, name="shiftb")
            nc.gpsimd.memset(shiftb[:], SHIFT)
            warm_exp = constp.tile([128, 1], F32, name="warm_exp")
            with nc.allow_low_precision("warmup junk"):
                nc.scalar.activation(
                    warm_exp[:], shiftb[:], EXPF, bias=shiftb[:], scale=SCALE
                )
            warm_ps = stps.tile([128, 512], F32, name="warm_ps", tag="stps")
            # 34 x 128-col then 10 x 64-col: the streak must run gaplessly
            # into the first data-dependent matmul (an idle gap resets the
            # HAM busy window), and the 64-col tail keeps the abutment jitter
            # under ~60ns against DMA-arrival variance.
            for _ in range(28):
                nc.tensor.matmul(
                    warm_ps[:, 0:128], dummy[:], dummy[:], start=True, stop=True
                )
            for _ in range(10):
                nc.tensor.matmul(
                    warm_ps[:, 0:64], dummy[:], dummy[:, 0:64], start=True, stop=True
                )

            # ---- consts ----
            ones_h = constp.tile([128, 128], F16, name="ones_h")
            nc.gpsimd.memset(ones_h[:], 1.0)
            ones8 = constp.tile([128, 256], F8, name="ones8")
            nc.gpsimd.memset(ones8[:], 1.0)

            pending = []

            def flush_one():
                if pending:
                    pending.pop(0)()

            def flush_all():
                while pending:
                    pending.pop(0)()

            def pe_pad(n):
                # dummy matmuls emitted where the in-order PE queue would
                # otherwise idle waiting on exp: keeps the HAM busy-streak
                # alive through the PE-sparse early q-blocks so the clock
                # un-throttles once and stays at 2.4 GHz.
                for _ in range(n):
                    nc.tensor.matmul(
                        warm_ps[:, 0:128], dummy[:], dummy[:], start=True, stop=True
                    )

            for b in range(BPC):
                qkv = qkv_tiles[b]
                v8 = v8_tiles[b]

                def kt_ap(j, qkv=qkv):
                    c = _kt_col(j * 128)
                    return qkv[:, c : c + 128]

                def qt_ap(c0, w, qkv=qkv):
                    c = _qt_col(c0)
                    return qkv[:, c : c + w]

                def vr_ap(j, qkv=qkv):
                    c = _vr_col(j * 128)
                    return qkv[:, c : c + 128]

                def v8_ap(t, v8=v8):
                    return v8[:, 256 * t : 256 * (t + 1)].rearrange(
                        "p (i m) -> p i m", i=2
                    )

                for qb in range(NQB):
                    n_full = 4 * qb
                    n_pairs = n_full // 2
                    q0 = qb * 512
                    # DVE-trick assignment per q-block: pair index in dve_set
                    if qb == 1:
                        dve_set = {1}
                    elif qb == 2:
                        dve_set = {1, 3}
                    elif qb == 3:
                        dve_set = {1, 4}
                    else:
                        dve_set = set()

                    ot = otps.tile([128, 512], F32, name="ot")
                    lp = lps.tile([128, 512], F32, name="lp", tag="lp")
                    final = b == BPC - 1 and qb == NQB - 1

                    def emit_evac(ot=ot, lp=lp, b=b, q0=q0, qb=qb, final=final):
                        ots = evacp.tile([128, 512], F16, name="ots")
                        lsb = lsbp.tile([1, 512], F32, name="lsb")
                        li = (b * NQB + qb) * 512
                        if final:
                            # ACT is idle after its last exp: route the
                            # rowsum row and half the output through it.
                            # Separate tiles let the ACT and DVE copies run
                            # concurrently, and the l/o DMAs split across
                            # both HWDGE rings so their issue overlaps.
                            ots2 = evacp.tile([128, 256], F16, name="ots2")
                            nc.scalar.copy(lsb[:], lp[0:1, :])
                            nc.scalar.dma_start(l_d[:, li : li + 512], lsb[:])
                            with nc.allow_low_precision("fp16 O^T ship"):
                                nc.scalar.copy(ots[:, 0:256], ot[:, 0:256])
                                nc.scalar.dma_start(
                                    o_d[b, :, q0 : q0 + 256], ots[:, 0:256]
                                )
                                nc.vector.tensor_copy(ots2[:], ot[:, 256:512])
                                nc.sync.dma_start(
                                    o_d[b, :, q0 + 256 : q0 + 512], ots2[:]
                                )
                        else:
                            nc.vector.tensor_copy(lsb[:], lp[0:1, :])
                            nc.sync.dma_start(l_d[:, li : li + 512], lsb[:])
                            with nc.allow_low_precision("fp16 O^T ship"):
                                nc.vector.tensor_copy(ots[:], ot[:])
                            nc.sync.dma_start(o_d[b, :, q0 : q0 + 512], ots[:])

                    # ---- diagonal groups first: their exact-fp16 exps are
                    # the slowest ACT work in the block, so they overlap the
                    # pair QK stream instead of sitting on the block's tail.
                    dgroups = [
                        ([(n_full, 0, 512, 0), (n_full + 1, 128, 384, 512)], 896),
                        (
                            [(n_full + 2, 256, 256, 0), (n_full + 3, 384, 128, 256)],
                            384,
                        ),
                    ]
                    for gi, (chunks, extent) in enumerate(dgroups):
                        st = stps.tile([128, 1024], F32, name="st", tag="stps")
                        for (j, qoff, width, col) in chunks:
                            nc.tensor.matmul(
                                st[:, col : col + width],
                                kt_ap(j),
                                qt_ap(q0 + qoff, width),
                                start=True,
                                stop=True,
                            )
                        pt = ptp.tile([128, 1024], F16, name="pt", tag="pt")
                        with nc.allow_low_precision("fp16 P within tolerance"):
                            nc.scalar.activation(
                                pt[:, 0:extent],
                                st[:, 0:extent],
                                EXPF,
                                bias=shiftb[:],
                                scale=SCALE,
                            )
                        # zero P where q < k (keep local col >= partition)
                        for (j, qoff, width, col) in chunks:
                            nc.gpsimd.affine_select(
                                out=pt[:, col : col + 128],
                                in_=pt[:, col : col + 128],
                                compare_op=mybir.AluOpType.is_ge,
                                fill=0.0,
                                base=0,
                                pattern=[[1, 128]],
                                channel_multiplier=-1,
                            )
                        if len(pending) >= 3:
                            flush_one()

                        def pv_diag(
                            chunks=chunks,
                            ot=ot,
                            lp=lp,
                            pt=pt,
                            gi=gi,
                            is_last=(gi == 1 and n_pairs == 0),
                            vr_ap=vr_ap,
                            emit_evac=emit_evac,
                        ):
                            for ci, (j, qoff, width, col) in enumerate(chunks):
                                nc.tensor.matmul(
                                    ot[:, qoff : qoff + width],
                                    vr_ap(j),
                                    pt[:, col : col + width],
                                    start=(gi == 0 and ci == 0),
                                    stop=(is_last and ci == 1),
                                )
                            for ci, (j, qoff, width, col) in enumerate(chunks):
                                nc.tensor.matmul(
                                    lp[0:1, qoff : qoff + width],
                                    ones_h[:, 0:1],
                                    pt[:, col : col + width],
                                    start=(gi == 0 and ci == 0),
                                    stop=(is_last and ci == 1),
                                )
                            if is_last:
                                emit_evac()

                        pending.append(pv_diag)

                    if b == 0 and qb == 0:
                        pe_pad(11)

                    # ---- full 2-chunk groups: fp8 end-to-end ----
                    for t in range(n_pairs):
                        st = stps.tile([128, 1024], F32, name="st", tag="stps")
                        for ci in range(2):
                            nc.tensor.matmul(
                                st[:, 512 * ci : 512 * ci + 512],
                                kt_ap(2 * t + ci),
                                qt_ap(q0, 512),
                                start=True,
                                stop=True,
                            )
                        pt8 = pt8p.tile([128, 1024], F8, name="pt8", tag="pt8")
                        if t in dve_set:
                            tmp = trickp.tile([128, 1024], I32, name="tmp", tag="tk")
                            with nc.allow_low_precision("exp2 bit-trick"):
                                nc.vector.tensor_scalar(
                                    tmp[:],
                                    st[:],
                                    TRICK_A,
                                    TRICK_B,
                                    MULT,
                                    ADD,
                                )
                                nc.vector.tensor_copy(pt8[:], tmp[:].bitcast(F32))
                        else:
                            with nc.allow_low_precision("fp8 P within tolerance"):
                                nc.scalar.activation(
                                    pt8[:],
                                    st[:],
                                    EXPF,
                                    bias=shiftb[:],
                                    scale=SCALE,
                                )
                        if len(pending) >= 3:
                            flush_one()

                        def pv_full(
                            t=t,
                            ot=ot,
                            lp=lp,
                            pt8=pt8,
                            v8_ap=v8_ap,
                            is_last=(t == n_pairs - 1),
                            final=final,
                            emit_evac=emit_evac,
                        ):
                            # M=1 rowsum: only row 0 of lp is ever read, and
                            # a 1-col weight makes the LDWEIGHTS ~free.  On
                            # the final group the rowsum goes first so the l
                            # evacuation overlaps the last PV matmul.
                            def rs():
                                nc.tensor.matmul(
                                    lp[0:1, :],
                                    ones8[:].rearrange("p (i m) -> p i m", i=2)[
                                        :, :, 0:1
                                    ],
                                    pt8[:].rearrange("p (i n) -> p i n", i=2),
                                    start=False,
                                    stop=is_last,
                                    perf_mode=DR,
                                )

                            def pv():
                                nc.tensor.matmul(
                                    ot[:],
                                    v8_ap(t),
                                    pt8[:].rearrange("p (i n) -> p i n", i=2),
                                    start=False,
                                    stop=is_last,
                                    perf_mode=DR,
                                )

                            if is_last and final:
                                rs()
                                pv()
                            else:
                                pv()
                                rs()
                            if is_last:
                                emit_evac()

                        pending.append(pv_full)

                    if b == 0 and qb == 1:
                        pe_pad(6)

            flush_all()
    nc.compile()
    return nc


_NC_CACHE = None


def _get_nc():
    global _NC_CACHE
    if _NC_CACHE is None:
        _NC_CACHE = build()
    return _NC_CACHE


def kernel(query, key, value, _trace=False):
    nc = _get_nc()
    F8NP = ml_dtypes.float8_e4m3
    in_maps = []
    for c in range(N_CORES):
        sl = slice(c * BPC, (c + 1) * BPC)
        q = np.asarray(query[sl], dtype=np.float32).astype(np.float16)
        k = np.asarray(key[sl], dtype=np.float32).astype(np.float16)
        v = np.asarray(value[sl], dtype=np.float32).astype(np.float16)
        x = np.empty((BPC, 128, 6144), np.float16)
        x8 = np.empty((BPC, 128, 1536), F8NP)
        for b in range(BPC):
            ktp = k[b].T
            qtp = q[b].T
            vrp = np.ascontiguousarray(
                v[b].reshape(16, 128, 128).transpose(1, 0, 2)
            ).reshape(128, 2048)
            x[b, :, 0:256] = ktp[:, 0:256]
            x[b, :, 256:768] = qtp[:, 0:512]
            x[b, :, 768:1024] = ktp[:, 256:512]
            x[b, :, 1024:1536] = vrp[:, 0:512]
            for i in range(1, 4):
                base = 1536 * i
                cs = slice(512 * i, 512 * (i + 1))
                x[b, :, base : base + 512] = ktp[:, cs]
                x[b, :, base + 512 : base + 1024] = qtp[:, cs]
                x[b, :, base + 1024 : base + 1536] = vrp[:, cs]
            x8[b] = vrp[:, 0:1536].astype(F8NP)
        in_maps.append({"x": x, "x8": x8})
    res = run_bass_kernel_spmd(
        nc, in_maps, core_ids=list(range(N_CORES)), trace=_trace
    )
    outs = []
    for c in range(N_CORES):
        o = res.results[c]["o"].astype(np.float32)
        l = res.results[c]["l"].reshape(BPC, S).astype(np.float32)
        outs.append(o.transpose(0, 2, 1) / l[:, :, None])
    out = np.ascontiguousarray(np.concatenate(outs, axis=0), dtype=np.float32)
    if _trace:
        return out, res
    return out


# revision 26
# speedup vs baseline: 1.0323x; 1.0323x over previous
"""Causal attention kernel for Trainium2 (Bass/Tile), 8-core SPMD.

Problem: B=16, S=2048, D=128 fp32 causal attention
    scores = Q @ K^T; scores -= INF*triu(k=1); attn = softmax(scores/sqrt(D));
    out = attn @ V.   Batch dim sharded across 8 cores, 2 batches per core.

v3 design (from v2 @ ~59.5us):
  - Startup: qkv piece-0 DMA issued first on the sync ring (cold DMA latency
    ~2.5us dominates the head); all remaining loads queue behind it in ring
    order (in-order per-queue service gives each piece near-solo bandwidth,
    replacing v2's ACT-ring burn-activation pacing).  Warmup is 20 x 128-col
    matmuls (fine-grained: blocks the first real MM by <=110ns) plus a 1-col
    exp to hoist the ACT table load.
  - Full (non-diagonal) k-chunk groups go fp8 end-to-end: exp writes
    float8e4 P' directly (no fp16 copy, no DVE cast), and BOTH the PV
    matmul and the rowsum run as fp8 DoubleRow (contract 256 = 2 chunks
    per 512-col stream): PE stream work drops ~5us/core vs fp16 PV.
    V rides a second fp8 input tensor (chunk-pair interleaved to match
    the DR weight AP).  Numerics: P and l use the same quantized P', so
    fp8 error largely cancels in O/l; numpy sim of this exact pipeline
    measures rel err 6.3e-3 (gate 2e-2).
  - exp is split across two engines: ACT (LUT exp) takes 7 of 12 full
    groups per batch + all diagonal groups; the DVE takes 5 via a
    Schraudolph exp2 bit-trick (one fused mul-add tensor_scalar fp32->int32
    PSUM->SBUF, then a bitcast fp32->fp8 copy).  Without this the ACT is
    the bottleneck at ~41us busy; split, both sit ~29us under the PE's
    ~34us.  Trick groups are always full groups (>=512 attended keys per
    row), where the ~4% sawtooth error cancels between numerator and
    denominator (sim: no measurable degradation over pure fp8).
  - Diagonal groups keep exact fp16 exp + fp16 PV/rowsum (rows with few
    attended keys can't absorb fp8/approx error), masked via GPSIMD
    affine_select after the exp as in v2.
  - l is shipped per q-block (8 small DMAs) instead of one big end-of-
    kernel DMA; PV emission is deferred two groups behind exp so the
    in-order PE queue always has QK work while both exp engines run.
Host: out = (O^T / l).T in fp64.
"""

import os

os.environ.setdefault("MYCRO_LOCAL_CACHE", "1")

import math

import numpy as np
import ml_dtypes

import concourse.bass as bass
import concourse.mybir as mybir
import concourse.tile as tile
from concourse import bacc
from concourse.bass_utils import run_bass_kernel_spmd

F32 = mybir.dt.float32
F16 = mybir.dt.float16
F8 = mybir.dt.float8e4
I32 = mybir.dt.int32
EXPF = mybir.ActivationFunctionType.Exp
DR = mybir.MatmulPerfMode.DoubleRow
MULT = mybir.AluOpType.mult
ADD = mybir.AluOpType.add

N_CORES = 8
B = 16
S = 2048
D = 128
BPC = B // N_CORES
SCALE = 1.0 / math.sqrt(float(D))
SHIFT = -2.0  # exp bias; cancels in O/l, keeps P' <= e^4 fp8-safe
NQB = S // 512
NCH = S // 128

LOG2E = 1.4426950408889634
TRICK_C = 0.0579  # Schraudolph bias: minimizes max rel err of 2^frac approx
TRICK_A = SCALE * LOG2E * (1 << 23)
TRICK_B = ((127.0 - TRICK_C) + SHIFT * LOG2E) * (1 << 23)

N_WARM_MM = 28

# packed qkv column map, 4 pieces per batch (one per q-block of demand):
#   piece i [1536*i : 1536*(i+1)] = k[512i:512(i+1)] | q[...] | v[...]
# piece 0 is special-cased as k[0:256] | q[0:512] | k[256:512] | v[0:512] so
# the head DMA [0:768] carries exactly the first diagonal group's operands.


def _kt_col(c):
    if c < 256:
        return c
    if c < 512:
        return 768 + (c - 256)
    return 1536 * (c // 512) + (c % 512)


def _qt_col(c):
    if c < 512:
        return 256 + c
    return 1536 * (c // 512) + 512 + (c % 512)


def _vr_col(c):
    return 1536 * (c // 512) + 1024 + (c % 512)


def build():
    nc = bacc.Bacc("TRN2", target_bir_lowering=False, debug=False, num_devices=N_CORES)
    x_d = nc.dram_tensor("x", [BPC, 128, 6144], F16, kind="ExternalInput")
    x8_d = nc.dram_tensor("x8", [BPC, 128, 1536], F8, kind="ExternalInput")
    o_d = nc.dram_tensor("o", [BPC, 128, S], F16, kind="ExternalOutput")
    l_d = nc.dram_tensor("l", [1, BPC * NQB * 512], F32, kind="ExternalOutput")

    with tile.TileContext(nc) as tc:
        with (
            tc.tile_pool(name="const", bufs=1) as constp,
            tc.tile_pool(name="qkv", bufs=2) as qkvp,
            tc.tile_pool(name="v8", bufs=2) as v8p,
            tc.tile_pool(name="pt", bufs=4) as ptp,
            tc.tile_pool(name="pt8", bufs=4) as pt8p,
            tc.tile_pool(name="trick", bufs=3) as trickp,
            tc.tile_pool(name="evac", bufs=3) as evacp,
            tc.tile_pool(name="lsb", bufs=2) as lsbp,
            tc.tile_pool(name="stps", bufs=3, space="PSUM") as stps,
            tc.tile_pool(name="otps", bufs=1, space="PSUM") as otps,
            tc.tile_pool(name="lps", bufs=1, space="PSUM") as lps,
        ):
            # ---- DMA-first startup: piece0 rides the cold ring latency ----
            qkv_tiles = [
                qkvp.tile([128, 6144], F16, name=f"qkv{b}") for b in range(BPC)
            ]
            v8_tiles = [
                v8p.tile([128, 1536], F8, name=f"v8_{b}") for b in range(BPC)
            ]
            # kt+qt of each q-block outrank later-needed vr slices: the
            # diag-first compute order makes each block's FIRST matmuls
            # depend on its kt piece, so those must never be behind a vr
            # transfer in the (in-order) ring.
            nc.sync.dma_start(qkv_tiles[0][:, 0:768], x_d[0, :, 0:768])
            nc.sync.dma_start(qkv_tiles[0][:, 768:1536], x_d[0, :, 768:1536])
            nc.sync.dma_start(qkv_tiles[0][:, 1536:3072], x_d[0, :, 1536:3072])
            nc.sync.dma_start(qkv_tiles[0][:, 3072:4096], x_d[0, :, 3072:4096])
            nc.sync.dma_start(v8_tiles[0][:], x8_d[0, :, :])
            nc.sync.dma_start(qkv_tiles[0][:, 4096:4608], x_d[0, :, 4096:4608])
            nc.sync.dma_start(qkv_tiles[0][:, 4608:6144], x_d[0, :, 4608:6144])
            if BPC > 1:
                nc.sync.dma_start(qkv_tiles[1][:, 0:3072], x_d[1, :, 0:3072])
                nc.sync.dma_start(qkv_tiles[1][:, 3072:6144], x_d[1, :, 3072:6144])
                nc.sync.dma_start(v8_tiles[1][:], x8_d[1, :, :])

            # ---- warmup: ACT table load + PE HAM ramp while DMAs fly ----
            # dummy is written by the DVE (whose post-boot user slot opens
            # ~1us before GPSIMD's) so the warmup matmul streak — and with
            # it the HAM busy-window credit — starts as early as possible.
            dummy = constp.tile([128, 128], F16, name="dummy")
            nc.vector.memset(dummy[:], 0.0)
            shiftb = constp.tile([128, 1], F32, name="shiftb")
            nc.gpsimd.memset(shiftb[:], SHIFT)
            warm_exp = constp.tile([128, 1], F32, name="warm_exp")
            with nc.allow_low_precision("warmup junk"):
                nc.scalar.activation(
                    warm_exp[:], shiftb[:], EXPF, bias=shiftb[:], scale=SCALE
                )
            warm_ps = stps.tile([128, 512], F32, name="warm_ps", tag="stps")
            # 34 x 128-col then 10 x 64-col: the streak must run gaplessly
            # into the first data-dependent matmul (an idle gap resets the
            # HAM busy window), and the 64-col tail keeps the abutment jitter
            # under ~60ns against DMA-arrival variance.
            for _ in range(28):
                nc.tensor.matmul(
                    warm_ps[:, 0:128], dummy[:], dummy[:], start=True, stop=True
                )
            for _ in range(10):
                nc.tensor.matmul(
                    warm_ps[:, 0:64], dummy[:], dummy[:, 0:64], start=True, stop=True
                )

            # ---- consts ----
            ones_h = constp.tile([128, 128], F16, name="ones_h")
            nc.gpsimd.memset(ones_h[:], 1.0)
            ones8 = constp.tile([128, 256], F8, name="ones8")
            nc.gpsimd.memset(ones8[:], 1.0)

            pending = []

            def flush_one():
                if pending:
                    pending.pop(0)()

            def flush_all():
                while pending:
                    pending.pop(0)()

            def pe_pad(n):
                # dummy matmuls emitted where the in-order PE queue would
                # otherwise idle waiting on exp: keeps the HAM busy-streak
                # alive through the PE-sparse early q-blocks so the clock
                # un-throttles once and stays at 2.4 GHz.
                for _ in range(n):
                    nc.tensor.matmul(
                        warm_ps[:, 0:128], dummy[:], dummy[:], start=True, stop=True
                    )

            for b in range(BPC):
                qkv = qkv_tiles[b]
                v8 = v8_tiles[b]

                def kt_ap(j, qkv=qkv):
                    c = _kt_col(j * 128)
                    return qkv[:, c : c + 128]

                def qt_ap(c0, w, qkv=qkv):
                    c = _qt_col(c0)
                    return qkv[:, c : c + w]

                def vr_ap(j, qkv=qkv):
                    c = _vr_col(j * 128)
                    return qkv[:, c : c + 128]

                def v8_ap(t, v8=v8):
                    return v8[:, 256 * t : 256 * (t + 1)].rearrange(
                        "p (i m) -> p i m", i=2
                    )

                for qb in range(NQB):
                    n_full = 4 * qb
                    n_pairs = n_full // 2
                    q0 = qb * 512
                    # DVE-trick assignment per q-block: pair index in dve_set
                    if qb == 1:
                        dve_set = {1}
                    elif qb == 2:
                        dve_set = {1, 3}
                    elif qb == 3:
                        dve_set = {1, 4}
                    else:
                        dve_set = set()

                    if b == 0 and qb == 2:
                        # insurance against the qb2 kt-piece DMA landing
                        # late: keep the HAM busy-streak alive
                        pe_pad(6)

                    ot = otps.tile([128, 512], F32, name="ot")
                    lp = lps.tile([128, 512], F32, name="lp", tag="lp")
                    final = b == BPC - 1 and qb == NQB - 1

                    def emit_evac(ot=ot, lp=lp, b=b, q0=q0, qb=qb, final=final):
                        ots = evacp.tile([128, 512], F16, name="ots")
                        lsb = lsbp.tile([1, 512], F32, name="lsb")
                        li = (b * NQB + qb) * 512
                        if final:
                            # ACT is idle after its last exp: route the
                            # rowsum row and half the output through it.
                            # Separate tiles let the ACT and DVE copies run
                            # concurrently, and the l/o DMAs split across
                            # both HWDGE rings so their issue overlaps.
                            ots2 = evacp.tile([128, 256], F16, name="ots2")
                            nc.scalar.copy(lsb[:], lp[0:1, :])
                            nc.scalar.dma_start(l_d[:, li : li + 512], lsb[:])
                            with nc.allow_low_precision("fp16 O^T ship"):
                                nc.scalar.copy(ots[:, 0:256], ot[:, 0:256])
                                nc.scalar.dma_start(
                                    o_d[b, :, q0 : q0 + 256], ots[:, 0:256]
                                )
                                nc.vector.tensor_copy(ots2[:], ot[:, 256:512])
                                nc.sync.dma_start(
                                    o_d[b, :, q0 + 256 : q0 + 512], ots2[:]
                                )
                        else:
                            nc.vector.tensor_copy(lsb[:], lp[0:1, :])
                            nc.sync.dma_start(l_d[:, li : li + 512], lsb[:])
                            with nc.allow_low_precision("fp16 O^T ship"):
                                nc.vector.tensor_copy(ots[:], ot[:])
                            nc.sync.dma_start(o_d[b, :, q0 : q0 + 512], ots[:])

                    # ---- diagonal groups first: their exact-fp16 exps are
                    # the slowest ACT work in the block, so they overlap the
                    # pair QK stream instead of sitting on the block's tail.
                    dgroups = [
                        ([(n_full, 0, 512, 0), (n_full + 1, 128, 384, 512)], 896),
                        (
                            [(n_full + 2, 256, 256, 0), (n_full + 3, 384, 128, 256)],
                            384,
                        ),
                    ]
                    for gi, (chunks, extent) in enumerate(dgroups):
                        st = stps.tile([128, 1024], F32, name="st", tag="stps")
                        for (j, qoff, width, col) in chunks:
                            nc.tensor.matmul(
                                st[:, col : col + width],
                                kt_ap(j),
                                qt_ap(q0 + qoff, width),
                                start=True,
                                stop=True,
                            )
                        pt = ptp.tile([128, 1024], F16, name="pt", tag="pt")
                        with nc.allow_low_precision("fp16 P within tolerance"):
                            nc.scalar.activation(
                                pt[:, 0:extent],
                                st[:, 0:extent],
                                EXPF,
                                bias=shiftb[:],
                                scale=SCALE,
                            )
                        # zero P where q < k (keep local col >= partition)
                        for (j, qoff, width, col) in chunks:
                            nc.gpsimd.affine_select(
                                out=pt[:, col : col + 128],
                                in_=pt[:, col : col + 128],
                                compare_op=mybir.AluOpType.is_ge,
                                fill=0.0,
                                base=0,
                                pattern=[[1, 128]],
                                channel_multiplier=-1,
                            )
                        if len(pending) >= 3:
                            flush_one()

                        def pv_diag(
                            chunks=chunks,
                            ot=ot,
                            lp=lp,
                            pt=pt,
                            gi=gi,
                            is_last=(gi == 1 and n_pairs == 0),
                            vr_ap=vr_ap,
                            emit_evac=emit_evac,
                        ):
                            for ci, (j, qoff, width, col) in enumerate(chunks):
                                nc.tensor.matmul(
                                    ot[:, qoff : qoff + width],
                                    vr_ap(j),
                                    pt[:, col : col + width],
                                    start=(gi == 0 and ci == 0),
                                    stop=(is_last and ci == 1),
                                )
                            for ci, (j, qoff, width, col) in enumerate(chunks):
                                nc.tensor.matmul(
                                    lp[0:1, qoff : qoff + width],
                                    ones_h[:, 0:1],
                                    pt[:, col : col + width],
                                    start=(gi == 0 and ci == 0),
                                    stop=(is_last and ci == 1),
                                )
                            if is_last:
                                emit_evac()

                        pending.append(pv_diag)

                    if b == 0 and qb == 0:
                        pe_pad(11)

                    # ---- full 2-chunk groups: fp8 end-to-end ----
                    for t in range(n_pairs):
                        st = stps.tile([128, 1024], F32, name="st", tag="stps")
                        for ci in range(2):
                            nc.tensor.matmul(
                                st[:, 512 * ci : 512 * ci + 512],
                                kt_ap(2 * t + ci),
                                qt_ap(q0, 512),
                                start=True,
                                stop=True,
                            )
                        pt8 = pt8p.tile([128, 1024], F8, name="pt8", tag="pt8")
                        if t in dve_set:
                            tmp = trickp.tile([128, 1024], I32, name="tmp", tag="tk")
                            with nc.allow_low_precision("exp2 bit-trick"):
                                nc.vector.tensor_scalar(
                                    tmp[:],
                                    st[:],
                                    TRICK_A,
                                    TRICK_B,
                                    MULT,
                                    ADD,
                                )
                                nc.vector.tensor_copy(pt8[:], tmp[:].bitcast(F32))
                        else:
                            with nc.allow_low_precision("fp8 P within tolerance"):
                                nc.scalar.activation(
                                    pt8[:],
                                    st[:],
                                    EXPF,
                                    bias=shiftb[:],
                                    scale=SCALE,
                                )
                        if len(pending) >= 3:
                            flush_one()

                        def pv_full(
                            t=t,
                            ot=ot,
                            lp=lp,
                            pt8=pt8,
                            v8_ap=v8_ap,
                            is_last=(t == n_pairs - 1),
                            final=final,
                            emit_evac=emit_evac,
                        ):
                            # M=1 rowsum: only row 0 of lp is ever read, and
                            # a 1-col weight makes the LDWEIGHTS ~free.  On
                            # the final group the rowsum goes first so the l
                            # evacuation overlaps the last PV matmul.
                            def rs():
                                nc.tensor.matmul(
                                    lp[0:1, :],
                                    ones8[:].rearrange("p (i m) -> p i m", i=2)[
                                        :, :, 0:1
                                    ],
                                    pt8[:].rearrange("p (i n) -> p i n", i=2),
                                    start=False,
                                    stop=is_last,
                                    perf_mode=DR,
                                )

                            def pv():
                                nc.tensor.matmul(
                                    ot[:],
                                    v8_ap(t),
                                    pt8[:].rearrange("p (i n) -> p i n", i=2),
                                    start=False,
                                    stop=is_last,
                                    perf_mode=DR,
                                )

                            if is_last and final:
                                rs()
                                pv()
                            else:
                                pv()
                                rs()
                            if is_last:
                                emit_evac()

                        pending.append(pv_full)

                    if b == 0 and qb == 1:
                        pe_pad(6)

            flush_all()
    nc.compile()
    return nc


_NC_CACHE = None


def _get_nc():
    global _NC_CACHE
    if _NC_CACHE is None:
        _NC_CACHE = build()
    return _NC_CACHE


def kernel(query, key, value, _trace=False):
    nc = _get_nc()
    F8NP = ml_dtypes.float8_e4m3
    in_maps = []
    for c in range(N_CORES):
        sl = slice(c * BPC, (c + 1) * BPC)
        q = np.asarray(query[sl], dtype=np.float32).astype(np.float16)
        k = np.asarray(key[sl], dtype=np.float32).astype(np.float16)
        v = np.asarray(value[sl], dtype=np.float32).astype(np.float16)
        x = np.empty((BPC, 128, 6144), np.float16)
        x8 = np.empty((BPC, 128, 1536), F8NP)
        for b in range(BPC):
            ktp = k[b].T
            qtp = q[b].T
            vrp = np.ascontiguousarray(
                v[b].reshape(16, 128, 128).transpose(1, 0, 2)
            ).reshape(128, 2048)
            x[b, :, 0:256] = ktp[:, 0:256]
            x[b, :, 256:768] = qtp[:, 0:512]
            x[b, :, 768:1024] = ktp[:, 256:512]
            x[b, :, 1024:1536] = vrp[:, 0:512]
            for i in range(1, 4):
                base = 1536 * i
                cs = slice(512 * i, 512 * (i + 1))
                x[b, :, base : base + 512] = ktp[:, cs]
                x[b, :, base + 512 : base + 1024] = qtp[:, cs]
                x[b, :, base + 1024 : base + 1536] = vrp[:, cs]
            x8[b] = vrp[:, 0:1536].astype(F8NP)
        in_maps.append({"x": x, "x8": x8})
    res = run_bass_kernel_spmd(
        nc, in_maps, core_ids=list(range(N_CORES)), trace=_trace
    )
    outs = []
    for c in range(N_CORES):
        o = res.results[c]["o"].astype(np.float32)
        l = res.results[c]["l"].reshape(BPC, S).astype(np.float32)
        outs.append(o.transpose(0, 2, 1) / l[:, :, None])
    out = np.ascontiguousarray(np.concatenate(outs, axis=0), dtype=np.float32)
    if _trace:
        return out, res
    return out
